# revision 4
# baseline (speedup 1.0000x reference)
"""Trainium2 Bass kernel: cross-attention block (1x1-conv projections + MHA).

Full computation (reference semantics, fp32 inputs):
    q = x @ Wq.T + bq;  k,v = context @ Wkv.T + bkv (split)
    per head: out_h = softmax(q_h @ k_h.T * scale) @ v_h
    out = concat_heads @ Wo.T + bo

Sharding: 8 cores = 4 batches x 2 head-groups (4 heads each).  Each core
computes its batch/head-group partial of the output projection; the host
sums the two head-group partials per batch (the "all-reduce") and adds bo.

Per-core kernel (n = m = 2048, d = 256, local inner e = 256), v2:
  - context path (cast/transpose/k,v-proj) is the only serial prefix; the
    query path (x transpose + q-proj) and the out-projections are emitted
    interleaved into the attention jj-loops so the PE fills its slack while
    the ACT engine (exp) paces the steady state.
  - qT/kT biases are applied on the PSUM->SBUF copy via tensor_scalar_add
    (bias is per-partition there); v bias keeps the K=1 rank-1 matmul.
  - v' = [v_h | 1] per head (65 cols): the attn@v matmul also yields the
    softmax denominator as row 64 for free.
  - normalization happens AFTER the per-head out-projection partials
    (n-on-partitions layout): denominator rows are DMA-gathered into a
    [4, 512] tile, PE-transposed to [128, 16], one 128-lane reciprocal,
    then 4 fused scalar_tensor_tensor ops combine the per-head partials.
    This removes the serial [1,512] reciprocals and gpsimd broadcasts that
    dominated block boundaries in v1.
  - PSUM: sim 2x[128,1024] + av 2x[128,512] + pob 2x[128,512] = 8 banks;
    phase A borrows the same pools so there is no inter-phase barrier.
Matmul operands are bf16; accumulation fp32 in PSUM; softmax stats fp32.
"""

import sys

if "/opt/trn_rl_repo" not in sys.path:
    sys.path.insert(0, "/opt/trn_rl_repo")

from contextlib import ExitStack

import ml_dtypes
import numpy as np

import concourse.bacc as bacc
import concourse.tile as tile
from concourse import mybir
from concourse.bass_utils import run_bass_kernel_spmd

f32 = mybir.dt.float32
bf16 = mybir.dt.bfloat16

B = 4          # global batch
N = 2048       # query sequence
MSEQ = 2048    # context sequence
D = 256        # query/context feature dim
HEADS = 8      # global heads
EH = 4         # heads per core (head-group)
DH = 64        # head dim
E = EH * DH    # per-core inner dim (256)
OD = 256       # output dim
SCALE = DH ** -0.5
NCORES = 8

NT = N // 128      # 16 query 128-tiles
MT = MSEQ // 128   # 16 context 128-tiles
KD = D // 128      # 2 contraction tiles over d
NB = N // 512      # 4 query 512-blocks

_CACHE = {}


def _build():
    nc = bacc.Bacc()
    x = nc.declare_dram_parameter("x", [N, D], f32, isOutput=False)
    cx = nc.declare_dram_parameter("cx", [MSEQ, D], f32, isOutput=False)
    wq = nc.declare_dram_parameter("wq", [D, E], bf16, isOutput=False)
    wk = nc.declare_dram_parameter("wk", [D, E], bf16, isOutput=False)
    wv = nc.declare_dram_parameter("wv", [D, E], bf16, isOutput=False)
    wo = nc.declare_dram_parameter("wo", [EH, DH, OD], bf16, isOutput=False)
    bqc = nc.declare_dram_parameter("bqc", [128, KD], f32, isOutput=False)
    bkc = nc.declare_dram_parameter("bkc", [128, KD], f32, isOutput=False)
    bv = nc.declare_dram_parameter("bv", [1, E], bf16, isOutput=False)
    cst = nc.declare_dram_parameter("cst", [128, 256], bf16, isOutput=False)
    idf = nc.declare_dram_parameter("idf", [128, 128], f32, isOutput=False)
    out = nc.declare_dram_parameter("out", [N, OD], f32, isOutput=True)

    f32r = mybir.dt.float32r

    with tile.TileContext(nc) as tc, ExitStack() as ctx:
        P = ctx.enter_context(tc.tile_pool(name="persist", bufs=1))
        PSS = ctx.enter_context(tc.tile_pool(name="psS", bufs=2, space="PSUM"))
        PSV = ctx.enter_context(tc.tile_pool(name="psV", bufs=2, space="PSUM"))
        POB = ctx.enter_context(tc.tile_pool(name="psO", bufs=2, space="PSUM"))
        EX = ctx.enter_context(tc.tile_pool(name="expp", bufs=6))
        SM = ctx.enter_context(tc.tile_pool(name="smallp", bufs=2))
        OS = ctx.enter_context(tc.tile_pool(name="outs", bufs=3))

        cst_sb = P.tile([128, 256], bf16)   # ones | bf16 identity
        nc.sync.dma_start(out=cst_sb, in_=cst[:, :])
        idf_sb = P.tile([128, 128], f32r)
        nc.sync.dma_start(out=idf_sb, in_=idf[:, :].bitcast(f32r))
        ones = cst_sb[:, 0:128]
        idb = cst_sb[:, 128:256]

        wq_sb = P.tile([128, KD, E], bf16)
        wk_sb = P.tile([128, KD, E], bf16)
        wv_sb = P.tile([128, KD, E], bf16)
        wo_sb = P.tile([64, EH, OD], bf16)
        nc.sync.dma_start(out=wq_sb, in_=wq.rearrange("(k p) e -> p k e", p=128))
        nc.sync.dma_start(out=wk_sb, in_=wk.rearrange("(k p) e -> p k e", p=128))
        nc.sync.dma_start(out=wv_sb, in_=wv.rearrange("(k p) e -> p k e", p=128))
        nc.sync.dma_start(out=wo_sb, in_=wo.rearrange("h p o -> p h o"))
        bqc_sb = P.tile([128, KD], f32)
        bkc_sb = P.tile([128, KD], f32)
        bv_sb = P.tile([1, E], bf16)
        nc.sync.dma_start(out=bqc_sb, in_=bqc[:, :])
        nc.sync.dma_start(out=bkc_sb, in_=bkc[:, :])
        nc.sync.dma_start(out=bv_sb, in_=bv[:, :])

        xT = P.tile([128, KD, N], bf16)     # x.T  (d on partitions)
        cT = P.tile([128, KD, MSEQ], bf16)  # ctx.T
        qT = P.tile([128, KD, N], bf16)     # q.T  (e on partitions)
        kT = P.tile([128, KD, MSEQ], bf16)  # k.T
        vS = P.tile([128, MT, EH, DH + 1], bf16)  # v' with ones column per head
        oTS = P.tile([65, EH, N], bf16)     # unnorm attn out + denom row 64

        xs = P.tile([128, NT, D], f32r)
        cs = P.tile([128, MT, D], f32r)
        xr = x.rearrange("(t p) d -> p t d", p=128).bitcast(f32r)
        cr = cx.rearrange("(t p) d -> p t d", p=128).bitcast(f32r)

        # ---------------- emission helpers --------------------------------
        def emit_transpose(src, dst, t):
            for k in range(KD):
                pt = POB.tile([128, 128], f32r, tag="pob", name="pt")
                nc.tensor.transpose(pt, src[:, t, k * 128:(k + 1) * 128], idf_sb)
                nc.vector.tensor_copy(dst[:, k, t * 128:(t + 1) * 128],
                                      pt[:, :].bitcast(f32))

        def emit_proj(w_sb, b_sb, src, dst, m, blk):
            # dst[:, m, blk*512:...] = (src.T @ w_sb)[:, m-slice] + bias col
            pq = PSS.tile([128, 512], f32, tag="sim", name="pq")
            for k in range(KD):
                nc.tensor.matmul(pq, w_sb[:, k, m * 128:(m + 1) * 128],
                                 src[:, k, blk * 512:(blk + 1) * 512],
                                 start=(k == 0), stop=(k == KD - 1))
            nc.vector.tensor_scalar_add(
                out=dst[:, m, blk * 512:(blk + 1) * 512], in0=pq,
                scalar1=b_sb[:, m:m + 1])

        def emit_vproj(mt):
            pv = PSS.tile([128, E], f32, tag="sim", name="pv")
            nc.tensor.matmul(pv, ones[0:1, 0:128], bv_sb[0:1, :],
                             start=True, stop=False)
            for k in range(KD):
                nc.tensor.matmul(pv, cT[:, k, mt * 128:(mt + 1) * 128],
                                 wv_sb[:, k, :], start=False, stop=(k == KD - 1))
            nc.vector.tensor_copy(
                vS[:, mt, :, 0:DH],
                pv.rearrange("p (h c) -> p h c", h=EH))

        def emit_xpath(ii):
            for t in range(4 * ii, 4 * ii + 4):
                emit_transpose(xs, xT, t)
            for m in range(KD):
                emit_proj(wq_sb, bqc_sb, xT, qT, m, ii)

        # rcp prep for block ii: dn rows -> [128, 16] psum -> reciprocal
        def emit_rcp(ii, dn_sb, rcp_sb):
            rp = POB.tile([128, 16], bf16, tag="pob", name="rp")
            for s in range(4):
                nc.tensor.transpose(
                    rp[:, 4 * s:4 * s + 4],
                    dn_sb[0:4, s * 128:(s + 1) * 128],
                    idb[0:4, 0:4])
            nc.vector.reciprocal(rcp_sb, rp[:, 0:16])

        def emit_outproj_nt(ii, nt, rcp_sb):
            # per-head partials, then fused scale-accumulate with 1/denom
            pobA = POB.tile([128, 512], f32, tag="pob", name="pobA")
            pobB = POB.tile([128, 512], f32, tag="pob", name="pobB")
            sl = slice(nt * 128, (nt + 1) * 128)
            for h in range(EH):
                dstp = (pobA if h < 2 else pobB)
                c0 = 256 * (h % 2)
                nc.tensor.matmul(dstp[:, c0:c0 + 256], oTS[0:64, h, sl],
                                 wo_sb[0:64, h, :], start=True, stop=True)
            c = 4 * (nt % 4)
            t0 = SM.tile([128, 256], f32, tag="t0", name="t0")
            t1 = SM.tile([128, 256], f32, tag="t1", name="t1")
            ot = OS.tile([128, 256], f32, tag="ot", name="ot")
            nc.vector.tensor_scalar_mul(
                out=t0, in0=pobA[:, 0:256], scalar1=rcp_sb[:, c + 0:c + 1])
            nc.vector.scalar_tensor_tensor(
                out=t1, in0=pobA[:, 256:512], scalar=rcp_sb[:, c + 1:c + 2],
                in1=t0, op0=mybir.AluOpType.mult, op1=mybir.AluOpType.add)
            nc.vector.scalar_tensor_tensor(
                out=t0, in0=pobB[:, 0:256], scalar=rcp_sb[:, c + 2:c + 3],
                in1=t1, op0=mybir.AluOpType.mult, op1=mybir.AluOpType.add)
            nc.vector.scalar_tensor_tensor(
                out=ot, in0=pobB[:, 256:512], scalar=rcp_sb[:, c + 3:c + 4],
                in1=t0, op0=mybir.AluOpType.mult, op1=mybir.AluOpType.add)
            nc.sync.dma_start(out=out[sl, :], in_=ot)

        # ---------------- context path (serial prefix) --------------------
        for t in range(MT):
            nc.sync.dma_start(out=cs[:, t, :], in_=cr[:, t, :])
        for t in range(NT):
            nc.sync.dma_start(out=xs[:, t, :], in_=xr[:, t, :])

        for t in range(MT):
            emit_transpose(cs, cT, t)
        for m in range(KD):
            for blk in range(MSEQ // 512):
                emit_proj(wk_sb, bkc_sb, cT, kT, m, blk)
        nc.vector.tensor_copy(
            vS[:, :, :, DH],
            cst_sb[:, 0:64].rearrange("p (a b) -> p a b", a=MT))
        for mt in range(MT):
            emit_vproj(mt)

        # query path for ii=0 must precede the first block
        emit_xpath(0)

        # ---------------- attention blocks --------------------------------
        dn_tiles = {}
        rcp_tiles = {}
        for ii in range(NB):
            dn_sb = SM.tile([4, 512], bf16, tag="dn", name=f"dn{ii}")
            dn_tiles[ii] = dn_sb
            for hp in range(2):
                h0, h1 = 2 * hp, 2 * hp + 1
                av0 = PSV.tile([128, 512], f32, tag="av", name="av0")
                av1 = PSV.tile([128, 512], f32, tag="av", name="av1")

                def emit_av(j2, e2, av0=av0, av1=av1, h0=h0, h1=h1):
                    nc.tensor.matmul(
                        av0[0:DH + 1, :], vS[:, j2, h0, :], e2[:, 0:512],
                        start=(j2 == 0), stop=(j2 == MT - 1),
                        skip_group_check=True)
                    nc.tensor.matmul(
                        av1[0:DH + 1, :], vS[:, j2, h1, :], e2[:, 512:1024],
                        start=(j2 == 0), stop=(j2 == MT - 1),
                        skip_group_check=True)

                # extra PE/DVE work injected into this block's jj loop
                extras = {}
                if hp == 0 and ii > 0:
                    pii = ii - 1
                    extras[2] = lambda pii=pii: emit_rcp(
                        pii, dn_tiles[pii], rcp_tiles[pii])
                    for nt_i in range(4):
                        extras[4 + 3 * nt_i] = (
                            lambda pii=pii, nt_i=nt_i: emit_outproj_nt(
                                pii, 4 * pii + nt_i, rcp_tiles[pii]))
                if hp == 1 and ii < NB - 1:
                    nxt = ii + 1
                    for t_i in range(4):
                        extras[1 + 2 * t_i] = (
                            lambda nxt=nxt, t_i=t_i: emit_transpose(
                                xs, xT, 4 * nxt + t_i))
                    extras[9] = lambda nxt=nxt: emit_proj(
                        wq_sb, bqc_sb, xT, qT, 0, nxt)
                    extras[11] = lambda nxt=nxt: emit_proj(
                        wq_sb, bqc_sb, xT, qT, 1, nxt)

                SKEW = 3
                exq = []
                for jj in range(MT):
                    sp = PSS.tile([128, 1024], f32, tag="sim", name="sp")
                    nc.tensor.matmul(
                        sp[:, 0:512],
                        kT[0:64, hp, jj * 128:(jj + 1) * 128],
                        qT[0:64, hp, ii * 512:(ii + 1) * 512],
                        start=True, stop=True)
                    nc.tensor.matmul(
                        sp[:, 512:1024],
                        kT[64:128, hp, jj * 128:(jj + 1) * 128],
                        qT[64:128, hp, ii * 512:(ii + 1) * 512],
                        start=True, stop=True)
                    ex = EX.tile([128, 1024], bf16, tag="exp", name="ex")
                    nc.scalar.activation(ex, sp, mybir.ActivationFunctionType.Exp)
                    exq.append((jj, ex))
                    if jj in extras:
                        extras[jj]()
                    if len(exq) > SKEW:
                        j2, e2 = exq.pop(0)
                        emit_av(j2, e2)
                for j2, e2 in exq:
                    emit_av(j2, e2)

                # denominators out (rows 64), unnormalized attn out to SBUF
                sli = slice(ii * 512, (ii + 1) * 512)
                nc.vector.tensor_copy(oTS[0:DH + 1, h0, sli], av0[0:DH + 1, :])
                nc.vector.tensor_copy(oTS[0:DH + 1, h1, sli], av1[0:DH + 1, :])
                nc.sync.dma_start(out=dn_sb[h0:h0 + 1, :],
                                  in_=oTS[DH:DH + 1, h0, sli])
                nc.sync.dma_start(out=dn_sb[h1:h1 + 1, :],
                                  in_=oTS[DH:DH + 1, h1, sli])
            rcp_tiles[ii] = SM.tile([128, 16], f32, tag="rcp", name=f"rcp{ii}")

        # tail: out-projection for the last ii
        lii = NB - 1
        emit_rcp(lii, dn_tiles[lii], rcp_tiles[lii])
        for nt_i in range(4):
            emit_outproj_nt(lii, 4 * lii + nt_i, rcp_tiles[lii])

    nc.finalize()
    return nc


def _get_nc():
    if "nc" not in _CACHE:
        _CACHE["nc"] = _build()
    return _CACHE["nc"]


def _make_in_maps(x, context, Wq, bq, Wkv, bkv, Wo, bo):
    f = np.float32
    b16 = ml_dtypes.bfloat16
    inner = HEADS * DH
    cstv = np.ones((128, 256), dtype=b16)
    cstv[:, 128:256] = np.eye(128, dtype=np.float32).astype(b16)
    in_maps = []
    for c in range(NCORES):
        b, g = divmod(c, 2)
        sl = slice(g * E, (g + 1) * E)
        slv = slice(inner + g * E, inner + (g + 1) * E)
        woT = np.ascontiguousarray(np.asarray(Wo)[:, sl].T, dtype=f)   # [E, OD]
        bq_l = (np.asarray(bq, dtype=f)[sl] * SCALE).reshape(KD, 128).T
        bk_l = np.asarray(bkv, dtype=f)[sl].reshape(KD, 128).T
        in_maps.append({
            "x": np.ascontiguousarray(x[b], dtype=f),
            "cx": np.ascontiguousarray(context[b], dtype=f),
            "wq": np.ascontiguousarray((np.asarray(Wq, dtype=f)[sl] * SCALE).T).astype(b16),
            "wk": np.ascontiguousarray(np.asarray(Wkv, dtype=f)[sl].T).astype(b16),
            "wv": np.ascontiguousarray(np.asarray(Wkv, dtype=f)[slv].T).astype(b16),
            "wo": woT.reshape(EH, DH, OD).astype(b16),
            "bqc": np.ascontiguousarray(bq_l),
            "bkc": np.ascontiguousarray(bk_l),
            "bv": np.asarray(bkv, dtype=f)[slv].reshape(1, E).astype(b16),
            "cst": cstv,
            "idf": np.eye(128, dtype=f),
        })
    return in_maps


def _run(in_maps, trace=False, tmpdir=None):
    nc = _get_nc()
    return run_bass_kernel_spmd(nc, in_maps, list(range(NCORES)),
                                trace=trace, tmpdir=tmpdir)


def kernel(x, context, Wq, bq, Wkv, bkv, Wo, bo):
    in_maps = _make_in_maps(x, context, Wq, bq, Wkv, bkv, Wo, bo)
    res = _run(in_maps)
    parts = [r["out"] for r in res.results]
    bo_f = np.asarray(bo, dtype=np.float32)
    full = np.stack([parts[2 * b] + parts[2 * b + 1] + bo_f for b in range(B)])
    return full.astype(np.float32)


# revision 11
# speedup vs baseline: 1.0554x; 1.0554x over previous
"""Trainium2 Bass kernel: cross-attention block (1x1-conv projections + MHA).

Full computation (reference semantics, fp32 inputs):
    q = x @ Wq.T + bq;  k,v = context @ Wkv.T + bkv (split)
    per head: out_h = softmax(q_h @ k_h.T * scale) @ v_h
    out = concat_heads @ Wo.T + bo

Sharding: 8 cores = 4 batches x 2 head-groups (4 heads each).  Each core
computes its batch/head-group partial of the output projection; the host
sums the two head-group partials per batch (the "all-reduce") and adds bo.

Per-core kernel (n = m = 2048, d = 256, local inner e = 256), v2:
  - context path (cast/transpose/k,v-proj) is the only serial prefix; the
    query path (x transpose + q-proj) and the out-projections are emitted
    interleaved into the attention jj-loops so the PE fills its slack while
    the ACT engine (exp) paces the steady state.
  - qT/kT biases are applied on the PSUM->SBUF copy via tensor_scalar_add
    (bias is per-partition there); v bias keeps the K=1 rank-1 matmul.
  - v' = [v_h | 1] per head (65 cols): the attn@v matmul also yields the
    softmax denominator as row 64 for free.
  - normalization happens AFTER the per-head out-projection partials
    (n-on-partitions layout): denominator rows are DMA-gathered into a
    [4, 512] tile, PE-transposed to [128, 16], one 128-lane reciprocal,
    then 4 fused scalar_tensor_tensor ops combine the per-head partials.
    This removes the serial [1,512] reciprocals and gpsimd broadcasts that
    dominated block boundaries in v1.
  - PSUM: sim 2x[128,1024] + av 2x[128,512] + pob 2x[128,512] = 8 banks;
    phase A borrows the same pools so there is no inter-phase barrier.
Matmul operands are bf16; accumulation fp32 in PSUM; softmax stats fp32.
"""

import sys

if "/opt/trn_rl_repo" not in sys.path:
    sys.path.insert(0, "/opt/trn_rl_repo")

from contextlib import ExitStack

import ml_dtypes
import numpy as np

import concourse.bacc as bacc
import concourse.tile as tile
from concourse import mybir
from concourse.bass_utils import run_bass_kernel_spmd

f32 = mybir.dt.float32
bf16 = mybir.dt.bfloat16

B = 4          # global batch
N = 2048       # query sequence
MSEQ = 2048    # context sequence
D = 256        # query/context feature dim
HEADS = 8      # global heads
EH = 4         # heads per core (head-group)
DH = 64        # head dim
E = EH * DH    # per-core inner dim (256)
OD = 256       # output dim
SCALE = DH ** -0.5
NCORES = 8

NT = N // 128      # 16 query 128-tiles
MT = MSEQ // 128   # 16 context 128-tiles
KD = D // 128      # 2 contraction tiles over d
NB = N // 512      # 4 query 512-blocks

_CACHE = {}


def _build():
    nc = bacc.Bacc()
    x = nc.declare_dram_parameter("x", [N, D], f32, isOutput=False)
    cx = nc.declare_dram_parameter("cx", [MSEQ, D], f32, isOutput=False)
    wq = nc.declare_dram_parameter("wq", [D, E], bf16, isOutput=False)
    wk = nc.declare_dram_parameter("wk", [D, E], bf16, isOutput=False)
    wv = nc.declare_dram_parameter("wv", [D, E], bf16, isOutput=False)
    wo = nc.declare_dram_parameter("wo", [EH, DH, OD], bf16, isOutput=False)
    bqc = nc.declare_dram_parameter("bqc", [128, KD], f32, isOutput=False)
    bkc = nc.declare_dram_parameter("bkc", [128, KD], f32, isOutput=False)
    bv = nc.declare_dram_parameter("bv", [128, E], bf16, isOutput=False)
    cst = nc.declare_dram_parameter("cst", [128, 256], bf16, isOutput=False)
    idf = nc.declare_dram_parameter("idf", [128, 128], f32, isOutput=False)
    out = nc.declare_dram_parameter("out", [N, OD], f32, isOutput=True)

    f32r = mybir.dt.float32r

    with tile.TileContext(nc) as tc, ExitStack() as ctx:
        P = ctx.enter_context(tc.tile_pool(name="persist", bufs=1))
        PSS = ctx.enter_context(tc.tile_pool(name="psS", bufs=2, space="PSUM"))
        PSV = ctx.enter_context(tc.tile_pool(name="psV", bufs=2, space="PSUM"))
        POB = ctx.enter_context(tc.tile_pool(name="psO", bufs=2, space="PSUM"))
        EX = ctx.enter_context(tc.tile_pool(name="expp", bufs=6))
        SM = ctx.enter_context(tc.tile_pool(name="smallp", bufs=2))
        OS = ctx.enter_context(tc.tile_pool(name="outs", bufs=3))

        cst_sb = P.tile([128, 256], bf16)   # ones | bf16 identity
        nc.sync.dma_start(out=cst_sb, in_=cst[:, :])
        idf_sb = P.tile([128, 128], f32r)
        nc.sync.dma_start(out=idf_sb, in_=idf[:, :].bitcast(f32r))
        ones = cst_sb[:, 0:128]
        idb = cst_sb[:, 128:256]

        wq_sb = P.tile([128, KD, E], bf16)
        wk_sb = P.tile([128, KD, E], bf16)
        wv_sb = P.tile([128, KD, E], bf16)
        wo_sb = P.tile([64, EH, OD], bf16)
        nc.sync.dma_start(out=wq_sb, in_=wq.rearrange("(k p) e -> p k e", p=128))
        nc.sync.dma_start(out=wk_sb, in_=wk.rearrange("(k p) e -> p k e", p=128))
        nc.sync.dma_start(out=wv_sb, in_=wv.rearrange("(k p) e -> p k e", p=128))
        nc.sync.dma_start(out=wo_sb, in_=wo.rearrange("h p o -> p h o"))
        bqc_sb = P.tile([128, KD], f32)
        bkc_sb = P.tile([128, KD], f32)
        bv_sb = P.tile([128, E], bf16)
        nc.sync.dma_start(out=bqc_sb, in_=bqc[:, :])
        nc.sync.dma_start(out=bkc_sb, in_=bkc[:, :])
        nc.sync.dma_start(out=bv_sb, in_=bv[:, :])

        xT = P.tile([128, KD, N], bf16)     # x.T  (d on partitions)
        cT = P.tile([128, KD, MSEQ], bf16)  # ctx.T
        qT = P.tile([128, KD, N], bf16)     # q.T  (e on partitions)
        kT = P.tile([128, KD, MSEQ], bf16)  # k.T
        vS = P.tile([128, MT, EH, DH + 1], bf16)  # v' with ones column per head
        oTS = P.tile([65, EH, N], bf16)     # unnorm attn out + denom row 64

        xs = P.tile([128, NT, D], f32r)
        cs = P.tile([128, MT, D], f32r)
        xr = x.rearrange("(t p) d -> p t d", p=128).bitcast(f32r)
        cr = cx.rearrange("(t p) d -> p t d", p=128).bitcast(f32r)

        # ---------------- emission helpers --------------------------------
        def emit_transpose(src, dst, t):
            for k in range(KD):
                pt = POB.tile([128, 128], f32r, tag="pob", name="pt")
                nc.tensor.transpose(pt, src[:, t, k * 128:(k + 1) * 128], idf_sb)
                nc.vector.tensor_copy(dst[:, k, t * 128:(t + 1) * 128],
                                      pt[:, :].bitcast(f32))

        def emit_proj(w_sb, b_sb, src, dst, m, blk):
            # dst[:, m, blk*512:...] = (src.T @ w_sb)[:, m-slice] + bias col
            pq = POB.tile([128, 512], f32, tag="pob", name="pq")
            for k in range(KD):
                nc.tensor.matmul(pq, w_sb[:, k, m * 128:(m + 1) * 128],
                                 src[:, k, blk * 512:(blk + 1) * 512],
                                 start=(k == 0), stop=(k == KD - 1))
            nc.vector.tensor_scalar_add(
                out=dst[:, m, blk * 512:(blk + 1) * 512], in0=pq,
                scalar1=b_sb[:, m:m + 1])

        def emit_vproj(mt):
            pv = POB.tile([128, E], f32, tag="pob", name="pv")
            for k in range(KD):
                nc.tensor.matmul(pv, cT[:, k, mt * 128:(mt + 1) * 128],
                                 wv_sb[:, k, :], start=(k == 0), stop=(k == KD - 1))
            nc.vector.scalar_tensor_tensor(
                out=vS[:, mt, :, 0:DH],
                in0=pv.rearrange("p (h c) -> p h c", h=EH),
                scalar=1.0,
                in1=bv_sb.rearrange("p (h c) -> p h c", h=EH),
                op0=mybir.AluOpType.mult, op1=mybir.AluOpType.add)

        def emit_xpath(ii):
            for t in range(4 * ii, 4 * ii + 4):
                emit_transpose(xs, xT, t)
            for m in range(KD):
                emit_proj(wq_sb, bqc_sb, xT, qT, m, ii)

        # rcp prep for block ii: dn rows -> [128, 16] psum -> reciprocal
        def emit_rcp(dn_sb, rcp_sb):
            rp = POB.tile([128, 16], bf16, tag="pob", name="rp")
            for s in range(4):
                nc.tensor.transpose(
                    rp[:, 4 * s:4 * s + 4],
                    dn_sb[0:4, s * 128:(s + 1) * 128],
                    idb[0:4, 0:4])
            nc.vector.reciprocal(rcp_sb, rp[:, 0:16])

        def emit_outproj_first(nt, rcp_sb, pp):
            # heads 0,1 partial: pp = pobA0*r0 + pobA1*r1
            pobA = POB.tile([128, 512], f32, tag="pob", name="pobA")
            sl = slice(nt * 128, (nt + 1) * 128)
            for hh in range(2):
                nc.tensor.matmul(pobA[:, 256 * hh:256 * hh + 256],
                                 oTS[0:64, hh, sl], wo_sb[0:64, hh, :],
                                 start=True, stop=True)
            c = 4 * (nt % 4)
            t0 = SM.tile([128, 256], f32, tag="t0", name="t0")
            nc.vector.tensor_scalar_mul(
                out=t0, in0=pobA[:, 0:256], scalar1=rcp_sb[:, c + 0:c + 1])
            nc.vector.scalar_tensor_tensor(
                out=pp, in0=pobA[:, 256:512], scalar=rcp_sb[:, c + 1:c + 2],
                in1=t0, op0=mybir.AluOpType.mult, op1=mybir.AluOpType.add)

        def emit_outproj_second(nt, rcp_sb, pp):
            # heads 2,3 + accumulate partial, then store
            pobB = POB.tile([128, 512], f32, tag="pob", name="pobB")
            sl = slice(nt * 128, (nt + 1) * 128)
            for hh in range(2):
                nc.tensor.matmul(pobB[:, 256 * hh:256 * hh + 256],
                                 oTS[0:64, 2 + hh, sl], wo_sb[0:64, 2 + hh, :],
                                 start=True, stop=True)
            c = 4 * (nt % 4)
            t1 = SM.tile([128, 256], f32, tag="t1", name="t1")
            ot = OS.tile([128, 256], f32, tag="ot", name="ot")
            nc.vector.scalar_tensor_tensor(
                out=t1, in0=pobB[:, 0:256], scalar=rcp_sb[:, c + 2:c + 3],
                in1=pp, op0=mybir.AluOpType.mult, op1=mybir.AluOpType.add)
            nc.vector.scalar_tensor_tensor(
                out=ot, in0=pobB[:, 256:512], scalar=rcp_sb[:, c + 3:c + 4],
                in1=t1, op0=mybir.AluOpType.mult, op1=mybir.AluOpType.add)
            nc.sync.dma_start(out=out[sl, :], in_=ot)

        # ---------------- context path (serial prefix) --------------------
        for t in range(MT):
            nc.sync.dma_start(out=cs[:, t, :], in_=cr[:, t, :])
        for t in range(NT):
            nc.sync.dma_start(out=xs[:, t, :], in_=xr[:, t, :])

        for t in range(MT):
            emit_transpose(cs, cT, t)
        emit_proj(wk_sb, bkc_sb, cT, kT, 0, 0)
        nc.vector.tensor_copy(
            vS[:, :, :, DH],
            cst_sb[:, 0:64].rearrange("p (a b) -> p a b", a=MT))
        for mt in range(10):
            emit_vproj(mt)
        for t in range(4):
            emit_transpose(xs, xT, t)
        emit_proj(wq_sb, bqc_sb, xT, qT, 0, 0)

        # ---------------- attention blocks --------------------------------
        dn_tiles = {}
        rcp_tiles = {}
        pp_tiles = {}
        for ii in range(NB):
            dn_sb = SM.tile([4, 512], bf16, tag="dn", name=f"dn{ii}")
            dn_tiles[ii] = dn_sb
            rcp_tiles[ii] = SM.tile([128, 16], f32, tag="rcp", name=f"rcp{ii}")
            if ii == NB - 1:
                # rows 2:4 are read (as junk) by the early rcp before block
                # (3,1) writes them; keep them finite for the reciprocal
                nc.vector.memset(dn_sb, 1.0)
            pp_tiles[ii] = [
                SM.tile([128, 256], f32, tag="pp", bufs=8, name=f"pp{ii}_{j}")
                for j in range(4)]
            for hp in range(2):
                h0, h1 = 2 * hp, 2 * hp + 1
                av0 = PSV.tile([128, 512], f32, tag="av", name="av0")
                av1 = PSV.tile([128, 512], f32, tag="av", name="av1")

                def emit_av(j2, e2, av0=av0, av1=av1, h0=h0, h1=h1):
                    nc.tensor.matmul(
                        av0[0:DH + 1, :], vS[:, j2, h0, :], e2[:, 0:512],
                        start=(j2 == 0), stop=(j2 == MT - 1),
                        skip_group_check=True)
                    nc.tensor.matmul(
                        av1[0:DH + 1, :], vS[:, j2, h1, :], e2[:, 512:1024],
                        start=(j2 == 0), stop=(j2 == MT - 1),
                        skip_group_check=True)

                # extra PE/DVE work injected into this block's jj loop
                extras = {}

                def add_extra(jj, fn, extras=extras):
                    extras.setdefault(jj, []).append(fn)

                if ii == 0 and hp == 0:
                    for b_i in range(1, 4):
                        add_extra(b_i - 1, lambda b_i=b_i: emit_proj(
                            wk_sb, bkc_sb, cT, kT, 0, b_i))
                    for mt_i in range(10, MT):
                        add_extra(mt_i - 7, lambda mt_i=mt_i: emit_vproj(mt_i))
                    for b_i in range(4):
                        add_extra(9 + b_i, lambda b_i=b_i: emit_proj(
                            wk_sb, bkc_sb, cT, kT, 1, b_i))
                    add_extra(13, lambda: emit_proj(wq_sb, bqc_sb, xT, qT, 1, 0))
                if hp == 0 and ii > 0:
                    pii = ii - 1
                    add_extra(2, lambda pii=pii: emit_rcp(
                        dn_tiles[pii], rcp_tiles[pii]))
                    for nt_i in range(4):
                        add_extra(4 + 3 * nt_i,
                                  lambda pii=pii, nt_i=nt_i: emit_outproj_first(
                                      4 * pii + nt_i, rcp_tiles[pii],
                                      pp_tiles[pii][nt_i]))
                        add_extra(5 + 3 * nt_i,
                                  lambda pii=pii, nt_i=nt_i: emit_outproj_second(
                                      4 * pii + nt_i, rcp_tiles[pii],
                                      pp_tiles[pii][nt_i]))
                if hp == 1 and ii < NB - 1:
                    nxt = ii + 1
                    for t_i in range(4):
                        add_extra(1 + 2 * t_i,
                                  lambda nxt=nxt, t_i=t_i: emit_transpose(
                                      xs, xT, 4 * nxt + t_i))
                    add_extra(9, lambda nxt=nxt: emit_proj(
                        wq_sb, bqc_sb, xT, qT, 0, nxt))
                    add_extra(11, lambda nxt=nxt: emit_proj(
                        wq_sb, bqc_sb, xT, qT, 1, nxt))
                if hp == 1 and ii == NB - 1:
                    add_extra(6, lambda: emit_rcp(dn_tiles[3], rcp_tiles[3]))
                    for nt_i in range(4):
                        add_extra(8 + 2 * nt_i,
                                  lambda nt_i=nt_i: emit_outproj_first(
                                      12 + nt_i, rcp_tiles[3],
                                      pp_tiles[3][nt_i]))

                SKEW = 3
                exq = []
                for jj in range(MT):
                    sp = PSS.tile([128, 1024], f32, tag="sim", name="sp")
                    nc.tensor.matmul(
                        sp[:, 0:512],
                        kT[0:64, hp, jj * 128:(jj + 1) * 128],
                        qT[0:64, hp, ii * 512:(ii + 1) * 512],
                        start=True, stop=True)
                    nc.tensor.matmul(
                        sp[:, 512:1024],
                        kT[64:128, hp, jj * 128:(jj + 1) * 128],
                        qT[64:128, hp, ii * 512:(ii + 1) * 512],
                        start=True, stop=True)
                    ex = EX.tile([128, 1024], bf16, tag="exp", name="ex")
                    nc.scalar.activation(ex, sp, mybir.ActivationFunctionType.Exp)
                    exq.append((jj, ex))
                    for fn in extras.get(jj, ()):
                        fn()
                    if len(exq) > SKEW:
                        j2, e2 = exq.pop(0)
                        emit_av(j2, e2)
                for j2, e2 in exq:
                    emit_av(j2, e2)

                # denominators + unnormalized attn out to SBUF, denom rows to dn
                sli = slice(ii * 512, (ii + 1) * 512)
                nc.vector.tensor_copy(oTS[0:DH + 1, h0, sli], av0[0:DH + 1, :])
                nc.vector.tensor_copy(oTS[0:DH + 1, h1, sli], av1[0:DH + 1, :])
                nc.sync.dma_start(out=dn_sb[h0:h0 + 1, :],
                                  in_=oTS[DH:DH + 1, h0, sli])
                nc.sync.dma_start(out=dn_sb[h1:h1 + 1, :],
                                  in_=oTS[DH:DH + 1, h1, sli])

        # tail: second halves (heads 2,3) of the last ii's out-projection
        rcp23 = SM.tile([128, 16], f32, tag="rcp", name="rcp23")
        emit_rcp(dn_tiles[3], rcp23)
        for nt_i in range(4):
            emit_outproj_second(12 + nt_i, rcp23, pp_tiles[3][nt_i])

    nc.finalize()
    return nc


def _get_nc():
    if "nc" not in _CACHE:
        _CACHE["nc"] = _build()
    return _CACHE["nc"]


def _make_in_maps(x, context, Wq, bq, Wkv, bkv, Wo, bo):
    f = np.float32
    b16 = ml_dtypes.bfloat16
    inner = HEADS * DH
    cstv = np.ones((128, 256), dtype=b16)
    cstv[:, 128:256] = np.eye(128, dtype=np.float32).astype(b16)
    in_maps = []
    for c in range(NCORES):
        b, g = divmod(c, 2)
        sl = slice(g * E, (g + 1) * E)
        slv = slice(inner + g * E, inner + (g + 1) * E)
        woT = np.ascontiguousarray(np.asarray(Wo)[:, sl].T, dtype=f)   # [E, OD]
        bq_l = (np.asarray(bq, dtype=f)[sl] * SCALE).reshape(KD, 128).T
        bk_l = np.asarray(bkv, dtype=f)[sl].reshape(KD, 128).T
        in_maps.append({
            "x": np.ascontiguousarray(x[b], dtype=f),
            "cx": np.ascontiguousarray(context[b], dtype=f),
            "wq": np.ascontiguousarray((np.asarray(Wq, dtype=f)[sl] * SCALE).T).astype(b16),
            "wk": np.ascontiguousarray(np.asarray(Wkv, dtype=f)[sl].T).astype(b16),
            "wv": np.ascontiguousarray(np.asarray(Wkv, dtype=f)[slv].T).astype(b16),
            "wo": woT.reshape(EH, DH, OD).astype(b16),
            "bqc": np.ascontiguousarray(bq_l),
            "bkc": np.ascontiguousarray(bk_l),
            "bv": np.tile(np.asarray(bkv, dtype=f)[slv].reshape(1, E),
                          (128, 1)).astype(b16),
            "cst": cstv,
            "idf": np.eye(128, dtype=f),
        })
    return in_maps


def _run(in_maps, trace=False, tmpdir=None):
    nc = _get_nc()
    return run_bass_kernel_spmd(nc, in_maps, list(range(NCORES)),
                                trace=trace, tmpdir=tmpdir)


def kernel(x, context, Wq, bq, Wkv, bkv, Wo, bo):
    in_maps = _make_in_maps(x, context, Wq, bq, Wkv, bkv, Wo, bo)
    res = _run(in_maps)
    parts = [r["out"] for r in res.results]
    bo_f = np.asarray(bo, dtype=np.float32)
    full = np.stack([parts[2 * b] + parts[2 * b + 1] + bo_f for b in range(B)])
    return full.astype(np.float32)


# revision 12
# speedup vs baseline: 1.0556x; 1.0002x over previous
"""Trainium2 Bass kernel: cross-attention block (1x1-conv projections + MHA).

Full computation (reference semantics, fp32 inputs):
    q = x @ Wq.T + bq;  k,v = context @ Wkv.T + bkv (split)
    per head: out_h = softmax(q_h @ k_h.T * scale) @ v_h
    out = concat_heads @ Wo.T + bo

Sharding: 8 cores = 4 batches x 2 head-groups (4 heads each).  Each core
computes its batch/head-group partial of the output projection; the host
sums the two head-group partials per batch (the "all-reduce") and adds bo.

Per-core kernel (n = m = 2048, d = 256, local inner e = 256), v2:
  - context path (cast/transpose/k,v-proj) is the only serial prefix; the
    query path (x transpose + q-proj) and the out-projections are emitted
    interleaved into the attention jj-loops so the PE fills its slack while
    the ACT engine (exp) paces the steady state.
  - qT/kT biases are applied on the PSUM->SBUF copy via tensor_scalar_add
    (bias is per-partition there); v bias keeps the K=1 rank-1 matmul.
  - v' = [v_h | 1] per head (65 cols): the attn@v matmul also yields the
    softmax denominator as row 64 for free.
  - normalization happens AFTER the per-head out-projection partials
    (n-on-partitions layout): denominator rows are DMA-gathered into a
    [4, 512] tile, PE-transposed to [128, 16], one 128-lane reciprocal,
    then 4 fused scalar_tensor_tensor ops combine the per-head partials.
    This removes the serial [1,512] reciprocals and gpsimd broadcasts that
    dominated block boundaries in v1.
  - PSUM: sim 2x[128,1024] + av 2x[128,512] + pob 2x[128,512] = 8 banks;
    phase A borrows the same pools so there is no inter-phase barrier.
Matmul operands are bf16; accumulation fp32 in PSUM; softmax stats fp32.
"""

import sys

if "/opt/trn_rl_repo" not in sys.path:
    sys.path.insert(0, "/opt/trn_rl_repo")

from contextlib import ExitStack

import ml_dtypes
import numpy as np

import concourse.bacc as bacc
import concourse.tile as tile
from concourse import mybir
from concourse.bass_utils import run_bass_kernel_spmd

f32 = mybir.dt.float32
bf16 = mybir.dt.bfloat16

B = 4          # global batch
N = 2048       # query sequence
MSEQ = 2048    # context sequence
D = 256        # query/context feature dim
HEADS = 8      # global heads
EH = 4         # heads per core (head-group)
DH = 64        # head dim
E = EH * DH    # per-core inner dim (256)
OD = 256       # output dim
SCALE = DH ** -0.5
NCORES = 8

NT = N // 128      # 16 query 128-tiles
MT = MSEQ // 128   # 16 context 128-tiles
KD = D // 128      # 2 contraction tiles over d
NB = N // 512      # 4 query 512-blocks

_CACHE = {}


def _build():
    nc = bacc.Bacc()
    x = nc.declare_dram_parameter("x", [N, D], bf16, isOutput=False)
    cx = nc.declare_dram_parameter("cx", [MSEQ, D], bf16, isOutput=False)
    wq = nc.declare_dram_parameter("wq", [D, E], bf16, isOutput=False)
    wk = nc.declare_dram_parameter("wk", [D, E], bf16, isOutput=False)
    wv = nc.declare_dram_parameter("wv", [D, E], bf16, isOutput=False)
    wo = nc.declare_dram_parameter("wo", [EH, DH, OD], bf16, isOutput=False)
    bqc = nc.declare_dram_parameter("bqc", [128, KD], f32, isOutput=False)
    bkc = nc.declare_dram_parameter("bkc", [128, KD], f32, isOutput=False)
    bv = nc.declare_dram_parameter("bv", [128, E], bf16, isOutput=False)
    cst = nc.declare_dram_parameter("cst", [128, 256], bf16, isOutput=False)
    out = nc.declare_dram_parameter("out", [N, OD], f32, isOutput=True)

    f32r = mybir.dt.float32r

    with tile.TileContext(nc) as tc, ExitStack() as ctx:
        P = ctx.enter_context(tc.tile_pool(name="persist", bufs=1))
        PSS = ctx.enter_context(tc.tile_pool(name="psS", bufs=2, space="PSUM"))
        PSV = ctx.enter_context(tc.tile_pool(name="psV", bufs=2, space="PSUM"))
        POB = ctx.enter_context(tc.tile_pool(name="psO", bufs=2, space="PSUM"))
        EX = ctx.enter_context(tc.tile_pool(name="expp", bufs=6))
        SM = ctx.enter_context(tc.tile_pool(name="smallp", bufs=2))
        OS = ctx.enter_context(tc.tile_pool(name="outs", bufs=3))

        cst_sb = P.tile([128, 256], bf16)   # ones | bf16 identity
        nc.sync.dma_start(out=cst_sb, in_=cst[:, :])
        ones = cst_sb[:, 0:128]
        idb = cst_sb[:, 128:256]

        wq_sb = P.tile([128, KD, E], bf16)
        wk_sb = P.tile([128, KD, E], bf16)
        wv_sb = P.tile([128, KD, E], bf16)
        wo_sb = P.tile([64, EH, OD], bf16)
        nc.sync.dma_start(out=wq_sb, in_=wq.rearrange("(k p) e -> p k e", p=128))
        nc.sync.dma_start(out=wk_sb, in_=wk.rearrange("(k p) e -> p k e", p=128))
        nc.sync.dma_start(out=wv_sb, in_=wv.rearrange("(k p) e -> p k e", p=128))
        nc.sync.dma_start(out=wo_sb, in_=wo.rearrange("h p o -> p h o"))
        bqc_sb = P.tile([128, KD], f32)
        bkc_sb = P.tile([128, KD], f32)
        bv_sb = P.tile([128, E], bf16)
        nc.sync.dma_start(out=bqc_sb, in_=bqc[:, :])
        nc.sync.dma_start(out=bkc_sb, in_=bkc[:, :])
        nc.sync.dma_start(out=bv_sb, in_=bv[:, :])

        xT = P.tile([128, KD, N], bf16)     # x.T  (d on partitions)
        cT = P.tile([128, KD, MSEQ], bf16)  # ctx.T
        qT = P.tile([128, KD, N], bf16)     # q.T  (e on partitions)
        kT = P.tile([128, KD, MSEQ], bf16)  # k.T
        vS = P.tile([128, MT, EH, DH + 1], bf16)  # v' with ones column per head
        oTS = P.tile([65, EH, N], bf16)     # unnorm attn out + denom row 64

        xs = P.tile([128, NT, D], bf16)
        cs = P.tile([128, MT, D], bf16)
        xr = x.rearrange("(t p) d -> p t d", p=128)
        cr = cx.rearrange("(t p) d -> p t d", p=128)

        # ---------------- emission helpers --------------------------------
        def emit_transpose(src, dst, t):
            for k in range(KD):
                pt = POB.tile([128, 128], bf16, tag="pob", name="pt")
                nc.tensor.transpose(pt, src[:, t, k * 128:(k + 1) * 128], idb)
                nc.vector.tensor_copy(dst[:, k, t * 128:(t + 1) * 128], pt)

        def emit_proj(w_sb, b_sb, src, dst, m, blk):
            # dst[:, m, blk*512:...] = (src.T @ w_sb)[:, m-slice] + bias col
            pq = POB.tile([128, 512], f32, tag="pob", name="pq")
            for k in range(KD):
                nc.tensor.matmul(pq, w_sb[:, k, m * 128:(m + 1) * 128],
                                 src[:, k, blk * 512:(blk + 1) * 512],
                                 start=(k == 0), stop=(k == KD - 1))
            nc.vector.tensor_scalar_add(
                out=dst[:, m, blk * 512:(blk + 1) * 512], in0=pq,
                scalar1=b_sb[:, m:m + 1])

        def emit_vproj(mt):
            pv = POB.tile([128, E], f32, tag="pob", name="pv")
            for k in range(KD):
                nc.tensor.matmul(pv, cT[:, k, mt * 128:(mt + 1) * 128],
                                 wv_sb[:, k, :], start=(k == 0), stop=(k == KD - 1))
            nc.vector.scalar_tensor_tensor(
                out=vS[:, mt, :, 0:DH],
                in0=pv.rearrange("p (h c) -> p h c", h=EH),
                scalar=1.0,
                in1=bv_sb.rearrange("p (h c) -> p h c", h=EH),
                op0=mybir.AluOpType.mult, op1=mybir.AluOpType.add)

        def emit_xpath(ii):
            for t in range(4 * ii, 4 * ii + 4):
                emit_transpose(xs, xT, t)
            for m in range(KD):
                emit_proj(wq_sb, bqc_sb, xT, qT, m, ii)

        # rcp prep for block ii: dn rows -> [128, 16] psum -> reciprocal
        def emit_rcp(dn_sb, rcp_sb):
            rp = POB.tile([128, 16], bf16, tag="pob", name="rp")
            for s in range(4):
                nc.tensor.transpose(
                    rp[:, 4 * s:4 * s + 4],
                    dn_sb[0:4, s * 128:(s + 1) * 128],
                    idb[0:4, 0:4])
            nc.vector.reciprocal(rcp_sb, rp[:, 0:16])

        def emit_outproj_first(nt, rcp_sb, pp):
            # heads 0,1 partial: pp = pobA0*r0 + pobA1*r1
            pobA = POB.tile([128, 512], f32, tag="pob", name="pobA")
            sl = slice(nt * 128, (nt + 1) * 128)
            for hh in range(2):
                nc.tensor.matmul(pobA[:, 256 * hh:256 * hh + 256],
                                 oTS[0:64, hh, sl], wo_sb[0:64, hh, :],
                                 start=True, stop=True)
            c = 4 * (nt % 4)
            t0 = SM.tile([128, 256], f32, tag="t0", name="t0")
            nc.vector.tensor_scalar_mul(
                out=t0, in0=pobA[:, 0:256], scalar1=rcp_sb[:, c + 0:c + 1])
            nc.vector.scalar_tensor_tensor(
                out=pp, in0=pobA[:, 256:512], scalar=rcp_sb[:, c + 1:c + 2],
                in1=t0, op0=mybir.AluOpType.mult, op1=mybir.AluOpType.add)

        def emit_outproj_second(nt, rcp_sb, pp):
            # heads 2,3 + accumulate partial, then store
            pobB = POB.tile([128, 512], f32, tag="pob", name="pobB")
            sl = slice(nt * 128, (nt + 1) * 128)
            for hh in range(2):
                nc.tensor.matmul(pobB[:, 256 * hh:256 * hh + 256],
                                 oTS[0:64, 2 + hh, sl], wo_sb[0:64, 2 + hh, :],
                                 start=True, stop=True)
            c = 4 * (nt % 4)
            t1 = SM.tile([128, 256], f32, tag="t1", name="t1")
            ot = OS.tile([128, 256], f32, tag="ot", name="ot")
            nc.vector.scalar_tensor_tensor(
                out=t1, in0=pobB[:, 0:256], scalar=rcp_sb[:, c + 2:c + 3],
                in1=pp, op0=mybir.AluOpType.mult, op1=mybir.AluOpType.add)
            nc.vector.scalar_tensor_tensor(
                out=ot, in0=pobB[:, 256:512], scalar=rcp_sb[:, c + 3:c + 4],
                in1=t1, op0=mybir.AluOpType.mult, op1=mybir.AluOpType.add)
            nc.sync.dma_start(out=out[sl, :], in_=ot)

        # ---------------- context path (serial prefix) --------------------
        for t in range(MT):
            nc.sync.dma_start(out=cs[:, t, :], in_=cr[:, t, :])
        for t in range(NT):
            nc.sync.dma_start(out=xs[:, t, :], in_=xr[:, t, :])

        for t in range(4):
            emit_transpose(cs, cT, t)
        for t in range(4):
            emit_transpose(xs, xT, t)
        emit_proj(wk_sb, bkc_sb, cT, kT, 0, 0)
        emit_proj(wq_sb, bqc_sb, xT, qT, 0, 0)
        nc.vector.tensor_copy(
            vS[:, :, :, DH],
            cst_sb[:, 0:64].rearrange("p (a b) -> p a b", a=MT))
        for mt in range(3):
            emit_vproj(mt)

        # ---------------- attention blocks --------------------------------
        dn_tiles = {}
        rcp_tiles = {}
        pp_tiles = {}
        for ii in range(NB):
            dn_sb = SM.tile([4, 512], bf16, tag="dn", name=f"dn{ii}")
            dn_tiles[ii] = dn_sb
            rcp_tiles[ii] = SM.tile([128, 16], f32, tag="rcp", name=f"rcp{ii}")
            if ii == NB - 1:
                # rows 2:4 are read (as junk) by the early rcp before block
                # (3,1) writes them; keep them finite for the reciprocal
                nc.vector.memset(dn_sb, 1.0)
            pp_tiles[ii] = [
                SM.tile([128, 256], f32, tag="pp", bufs=8, name=f"pp{ii}_{j}")
                for j in range(4)]
            for hp in range(2):
                h0, h1 = 2 * hp, 2 * hp + 1
                av0 = PSV.tile([128, 512], f32, tag="av", name="av0")
                av1 = PSV.tile([128, 512], f32, tag="av", name="av1")

                def emit_av(j2, e2, av0=av0, av1=av1, h0=h0, h1=h1):
                    nc.tensor.matmul(
                        av0[0:DH + 1, :], vS[:, j2, h0, :], e2[:, 0:512],
                        start=(j2 == 0), stop=(j2 == MT - 1),
                        skip_group_check=True)
                    nc.tensor.matmul(
                        av1[0:DH + 1, :], vS[:, j2, h1, :], e2[:, 512:1024],
                        start=(j2 == 0), stop=(j2 == MT - 1),
                        skip_group_check=True)

                # extra PE/DVE work injected into this block's jj loop
                extras = {}

                def add_extra(jj, fn, extras=extras):
                    extras.setdefault(jj, []).append(fn)

                if ii == 0 and hp == 0:
                    for t_i in range(4, MT):
                        add_extra((t_i - 4) // 2,
                                  lambda t_i=t_i: emit_transpose(cs, cT, t_i))
                    add_extra(2, lambda: emit_proj(wk_sb, bkc_sb, cT, kT, 0, 1))
                    add_extra(5, lambda: emit_proj(wk_sb, bkc_sb, cT, kT, 0, 2))
                    add_extra(9, lambda: emit_proj(wk_sb, bkc_sb, cT, kT, 0, 3))
                    for mt_i in range(3, MT):
                        add_extra(mt_i - 3, lambda mt_i=mt_i: emit_vproj(mt_i))
                    add_extra(11, lambda: emit_proj(wk_sb, bkc_sb, cT, kT, 1, 0))
                    add_extra(13, lambda: emit_proj(wq_sb, bqc_sb, xT, qT, 1, 0))
                if ii == 0 and hp == 1:
                    for b_i in range(1, 4):
                        add_extra(b_i - 1, lambda b_i=b_i: emit_proj(
                            wk_sb, bkc_sb, cT, kT, 1, b_i))
                if hp == 0 and ii > 0:
                    pii = ii - 1
                    add_extra(2, lambda pii=pii: emit_rcp(
                        dn_tiles[pii], rcp_tiles[pii]))
                    for nt_i in range(4):
                        add_extra(4 + 3 * nt_i,
                                  lambda pii=pii, nt_i=nt_i: emit_outproj_first(
                                      4 * pii + nt_i, rcp_tiles[pii],
                                      pp_tiles[pii][nt_i]))
                        add_extra(5 + 3 * nt_i,
                                  lambda pii=pii, nt_i=nt_i: emit_outproj_second(
                                      4 * pii + nt_i, rcp_tiles[pii],
                                      pp_tiles[pii][nt_i]))
                if hp == 1 and ii < NB - 1:
                    nxt = ii + 1
                    for t_i in range(4):
                        add_extra(1 + 2 * t_i,
                                  lambda nxt=nxt, t_i=t_i: emit_transpose(
                                      xs, xT, 4 * nxt + t_i))
                    add_extra(9, lambda nxt=nxt: emit_proj(
                        wq_sb, bqc_sb, xT, qT, 0, nxt))
                    add_extra(11, lambda nxt=nxt: emit_proj(
                        wq_sb, bqc_sb, xT, qT, 1, nxt))
                if hp == 1 and ii == NB - 1:
                    add_extra(6, lambda: emit_rcp(dn_tiles[3], rcp_tiles[3]))
                    for nt_i in range(4):
                        add_extra(8 + 2 * nt_i,
                                  lambda nt_i=nt_i: emit_outproj_first(
                                      12 + nt_i, rcp_tiles[3],
                                      pp_tiles[3][nt_i]))

                SKEW = 3
                exq = []
                for jj in range(MT):
                    sp = PSS.tile([128, 1024], f32, tag="sim", name="sp")
                    nc.tensor.matmul(
                        sp[:, 0:512],
                        kT[0:64, hp, jj * 128:(jj + 1) * 128],
                        qT[0:64, hp, ii * 512:(ii + 1) * 512],
                        start=True, stop=True)
                    nc.tensor.matmul(
                        sp[:, 512:1024],
                        kT[64:128, hp, jj * 128:(jj + 1) * 128],
                        qT[64:128, hp, ii * 512:(ii + 1) * 512],
                        start=True, stop=True)
                    ex = EX.tile([128, 1024], bf16, tag="exp", name="ex")
                    nc.scalar.activation(ex, sp, mybir.ActivationFunctionType.Exp)
                    exq.append((jj, ex))
                    for fn in extras.get(jj, ()):
                        fn()
                    if len(exq) > SKEW:
                        j2, e2 = exq.pop(0)
                        emit_av(j2, e2)
                for j2, e2 in exq:
                    emit_av(j2, e2)

                # denominators + unnormalized attn out to SBUF, denom rows to dn
                sli = slice(ii * 512, (ii + 1) * 512)
                nc.vector.tensor_copy(oTS[0:DH + 1, h0, sli], av0[0:DH + 1, :])
                nc.vector.tensor_copy(oTS[0:DH + 1, h1, sli], av1[0:DH + 1, :])
                nc.sync.dma_start(out=dn_sb[h0:h0 + 1, :],
                                  in_=oTS[DH:DH + 1, h0, sli])
                nc.sync.dma_start(out=dn_sb[h1:h1 + 1, :],
                                  in_=oTS[DH:DH + 1, h1, sli])

        # tail: second halves (heads 2,3) of the last ii's out-projection
        rcp23 = SM.tile([128, 16], f32, tag="rcp", name="rcp23")
        emit_rcp(dn_tiles[3], rcp23)
        for nt_i in range(4):
            emit_outproj_second(12 + nt_i, rcp23, pp_tiles[3][nt_i])

    nc.finalize()
    return nc


def _get_nc():
    if "nc" not in _CACHE:
        _CACHE["nc"] = _build()
    return _CACHE["nc"]


def _make_in_maps(x, context, Wq, bq, Wkv, bkv, Wo, bo):
    f = np.float32
    b16 = ml_dtypes.bfloat16
    inner = HEADS * DH
    cstv = np.ones((128, 256), dtype=b16)
    cstv[:, 128:256] = np.eye(128, dtype=np.float32).astype(b16)
    in_maps = []
    for c in range(NCORES):
        b, g = divmod(c, 2)
        sl = slice(g * E, (g + 1) * E)
        slv = slice(inner + g * E, inner + (g + 1) * E)
        woT = np.ascontiguousarray(np.asarray(Wo)[:, sl].T, dtype=f)   # [E, OD]
        bq_l = (np.asarray(bq, dtype=f)[sl] * SCALE).reshape(KD, 128).T
        bk_l = np.asarray(bkv, dtype=f)[sl].reshape(KD, 128).T
        in_maps.append({
            "x": np.ascontiguousarray(np.asarray(x[b], dtype=f)).astype(b16),
            "cx": np.ascontiguousarray(np.asarray(context[b], dtype=f)).astype(b16),
            "wq": np.ascontiguousarray((np.asarray(Wq, dtype=f)[sl] * SCALE).T).astype(b16),
            "wk": np.ascontiguousarray(np.asarray(Wkv, dtype=f)[sl].T).astype(b16),
            "wv": np.ascontiguousarray(np.asarray(Wkv, dtype=f)[slv].T).astype(b16),
            "wo": woT.reshape(EH, DH, OD).astype(b16),
            "bqc": np.ascontiguousarray(bq_l),
            "bkc": np.ascontiguousarray(bk_l),
            "bv": np.tile(np.asarray(bkv, dtype=f)[slv].reshape(1, E),
                          (128, 1)).astype(b16),
            "cst": cstv,
        })
    return in_maps


def _run(in_maps, trace=False, tmpdir=None):
    nc = _get_nc()
    return run_bass_kernel_spmd(nc, in_maps, list(range(NCORES)),
                                trace=trace, tmpdir=tmpdir)


def kernel(x, context, Wq, bq, Wkv, bkv, Wo, bo):
    in_maps = _make_in_maps(x, context, Wq, bq, Wkv, bkv, Wo, bo)
    res = _run(in_maps)
    parts = [r["out"] for r in res.results]
    bo_f = np.asarray(bo, dtype=np.float32)
    full = np.stack([parts[2 * b] + parts[2 * b + 1] + bo_f for b in range(B)])
    return full.astype(np.float32)


# revision 13
# speedup vs baseline: 1.0962x; 1.0384x over previous
"""Trainium2 Bass kernel: cross-attention block (1x1-conv projections + MHA).

Full computation (reference semantics, fp32 inputs):
    q = x @ Wq.T + bq;  k,v = context @ Wkv.T + bkv (split)
    per head: out_h = softmax(q_h @ k_h.T * scale) @ v_h
    out = concat_heads @ Wo.T + bo

Sharding: 8 cores = 4 batches x 2 head-groups (4 heads each).  Each core
computes its batch/head-group partial of the output projection; the host
sums the two head-group partials per batch (the "all-reduce") and adds bo.

Per-core kernel (n = m = 2048, d = 256, local inner e = 256), v2:
  - context path (cast/transpose/k,v-proj) is the only serial prefix; the
    query path (x transpose + q-proj) and the out-projections are emitted
    interleaved into the attention jj-loops so the PE fills its slack while
    the ACT engine (exp) paces the steady state.
  - qT/kT biases are applied on the PSUM->SBUF copy via tensor_scalar_add
    (bias is per-partition there); v bias keeps the K=1 rank-1 matmul.
  - v' = [v_h | 1] per head (65 cols): the attn@v matmul also yields the
    softmax denominator as row 64 for free.
  - normalization happens AFTER the per-head out-projection partials
    (n-on-partitions layout): denominator rows are DMA-gathered into a
    [4, 512] tile, PE-transposed to [128, 16], one 128-lane reciprocal,
    then 4 fused scalar_tensor_tensor ops combine the per-head partials.
    This removes the serial [1,512] reciprocals and gpsimd broadcasts that
    dominated block boundaries in v1.
  - PSUM: sim 2x[128,1024] + av 2x[128,512] + pob 2x[128,512] = 8 banks;
    phase A borrows the same pools so there is no inter-phase barrier.
Matmul operands are bf16; accumulation fp32 in PSUM; softmax stats fp32.
"""

import sys

if "/opt/trn_rl_repo" not in sys.path:
    sys.path.insert(0, "/opt/trn_rl_repo")

from contextlib import ExitStack

import ml_dtypes
import numpy as np

import concourse.bacc as bacc
import concourse.tile as tile
from concourse import mybir
from concourse.bass_utils import run_bass_kernel_spmd

f32 = mybir.dt.float32
bf16 = mybir.dt.bfloat16

B = 4          # global batch
N = 2048       # query sequence
MSEQ = 2048    # context sequence
D = 256        # query/context feature dim
HEADS = 8      # global heads
EH = 4         # heads per core (head-group)
DH = 64        # head dim
E = EH * DH    # per-core inner dim (256)
OD = 256       # output dim
SCALE = DH ** -0.5
NCORES = 8

NT = N // 128      # 16 query 128-tiles
MT = MSEQ // 128   # 16 context 128-tiles
KD = D // 128      # 2 contraction tiles over d
NB = N // 512      # 4 query 512-blocks

_CACHE = {}


def _build():
    nc = bacc.Bacc()
    x = nc.declare_dram_parameter("x", [N, D], bf16, isOutput=False)
    cx = nc.declare_dram_parameter("cx", [MSEQ, D], bf16, isOutput=False)
    wq = nc.declare_dram_parameter("wq", [D, E], bf16, isOutput=False)
    wk = nc.declare_dram_parameter("wk", [D, E], bf16, isOutput=False)
    wv = nc.declare_dram_parameter("wv", [D, E], bf16, isOutput=False)
    wo = nc.declare_dram_parameter("wo", [EH, DH, OD], bf16, isOutput=False)
    bqc = nc.declare_dram_parameter("bqc", [128, KD], f32, isOutput=False)
    bkc = nc.declare_dram_parameter("bkc", [128, KD], f32, isOutput=False)
    bv = nc.declare_dram_parameter("bv", [128, E], bf16, isOutput=False)
    cst = nc.declare_dram_parameter("cst", [128, 256], bf16, isOutput=False)
    out = nc.declare_dram_parameter("out", [N, OD], f32, isOutput=True)

    f32r = mybir.dt.float32r

    with tile.TileContext(nc) as tc, ExitStack() as ctx:
        P = ctx.enter_context(tc.tile_pool(name="persist", bufs=1))
        PSS = ctx.enter_context(tc.tile_pool(name="psS", bufs=2, space="PSUM"))
        PSV = ctx.enter_context(tc.tile_pool(name="psV", bufs=2, space="PSUM"))
        POB = ctx.enter_context(tc.tile_pool(name="psO", bufs=2, space="PSUM"))
        EX = ctx.enter_context(tc.tile_pool(name="expp", bufs=6))
        SM = ctx.enter_context(tc.tile_pool(name="smallp", bufs=2))
        OS = ctx.enter_context(tc.tile_pool(name="outs", bufs=3))

        cst_sb = P.tile([128, 256], bf16)   # ones | bf16 identity
        nc.sync.dma_start(out=cst_sb, in_=cst[:, :])
        ones = cst_sb[:, 0:128]
        idb = cst_sb[:, 128:256]

        wq_sb = P.tile([128, KD, E], bf16)
        wk_sb = P.tile([128, KD, E], bf16)
        wv_sb = P.tile([128, KD, E], bf16)
        wo_sb = P.tile([64, EH, OD], bf16)
        bqc_sb = P.tile([128, KD], f32)
        bkc_sb = P.tile([128, KD], f32)
        bv_sb = P.tile([128, E], bf16)

        xT = P.tile([128, KD, N], bf16)     # x.T  (d on partitions)
        cT = P.tile([128, KD, MSEQ], bf16)  # ctx.T
        qT = P.tile([128, KD, N], bf16)     # q.T  (e on partitions)
        kT = P.tile([128, KD, MSEQ], bf16)  # k.T
        vS = P.tile([128, MT, EH, DH + 1], bf16)  # v' with ones column per head
        oTS = P.tile([65, EH, N], bf16)     # unnorm attn out + denom row 64

        xs = P.tile([128, NT, D], bf16)
        cs = P.tile([128, MT, D], bf16)
        xr = x.rearrange("(t p) d -> p t d", p=128)
        cr = cx.rearrange("(t p) d -> p t d", p=128)

        # first staging tiles before the bulk weight loads: the transpose
        # chain is the critical path at kernel start
        for t in range(8):
            nc.sync.dma_start(out=cs[:, t, :], in_=cr[:, t, :])
        for t in range(4):
            nc.sync.dma_start(out=xs[:, t, :], in_=xr[:, t, :])
        nc.sync.dma_start(out=wk_sb, in_=wk.rearrange("(k p) e -> p k e", p=128))
        nc.sync.dma_start(out=wq_sb, in_=wq.rearrange("(k p) e -> p k e", p=128))
        nc.sync.dma_start(out=bqc_sb, in_=bqc[:, :])
        nc.sync.dma_start(out=bkc_sb, in_=bkc[:, :])
        nc.sync.dma_start(out=wv_sb, in_=wv.rearrange("(k p) e -> p k e", p=128))
        nc.sync.dma_start(out=bv_sb, in_=bv[:, :])
        nc.sync.dma_start(out=wo_sb, in_=wo.rearrange("h p o -> p h o"))

        # ---------------- emission helpers --------------------------------
        def emit_transpose(src, dst, t):
            for k in range(KD):
                pt = POB.tile([128, 128], bf16, tag="pob", name="pt")
                nc.tensor.transpose(pt, src[:, t, k * 128:(k + 1) * 128], idb)
                nc.vector.tensor_copy(dst[:, k, t * 128:(t + 1) * 128], pt)

        def emit_proj(w_sb, b_sb, src, dst, m, blk):
            # dst[:, m, blk*512:...] = (src.T @ w_sb)[:, m-slice] + bias col
            pq = POB.tile([128, 512], f32, tag="pob", name="pq")
            for k in range(KD):
                nc.tensor.matmul(pq, w_sb[:, k, m * 128:(m + 1) * 128],
                                 src[:, k, blk * 512:(blk + 1) * 512],
                                 start=(k == 0), stop=(k == KD - 1))
            nc.vector.tensor_scalar_add(
                out=dst[:, m, blk * 512:(blk + 1) * 512], in0=pq,
                scalar1=b_sb[:, m:m + 1])

        def emit_vproj(mt):
            pv = POB.tile([128, E], f32, tag="pob", name="pv")
            for k in range(KD):
                nc.tensor.matmul(pv, cT[:, k, mt * 128:(mt + 1) * 128],
                                 wv_sb[:, k, :], start=(k == 0), stop=(k == KD - 1))
            nc.vector.scalar_tensor_tensor(
                out=vS[:, mt, :, 0:DH],
                in0=pv.rearrange("p (h c) -> p h c", h=EH),
                scalar=1.0,
                in1=bv_sb.rearrange("p (h c) -> p h c", h=EH),
                op0=mybir.AluOpType.mult, op1=mybir.AluOpType.add)

        def emit_xpath(ii):
            for t in range(4 * ii, 4 * ii + 4):
                emit_transpose(xs, xT, t)
            for m in range(KD):
                emit_proj(wq_sb, bqc_sb, xT, qT, m, ii)

        # rcp prep for block ii: dn rows -> [128, 16] psum -> reciprocal
        def emit_rcp(dn_sb, rcp_sb):
            rp = POB.tile([128, 16], bf16, tag="pob", name="rp")
            for s in range(4):
                nc.tensor.transpose(
                    rp[:, 4 * s:4 * s + 4],
                    dn_sb[0:4, s * 128:(s + 1) * 128],
                    idb[0:4, 0:4])
            nc.vector.reciprocal(rcp_sb, rp[:, 0:16])

        def emit_outproj_first(nt, rcp_sb, pp):
            # heads 0,1 partial: pp = pobA0*r0 + pobA1*r1
            pobA = POB.tile([128, 512], f32, tag="pob", name="pobA")
            sl = slice(nt * 128, (nt + 1) * 128)
            for hh in range(2):
                nc.tensor.matmul(pobA[:, 256 * hh:256 * hh + 256],
                                 oTS[0:64, hh, sl], wo_sb[0:64, hh, :],
                                 start=True, stop=True)
            c = 4 * (nt % 4)
            t0 = SM.tile([128, 256], f32, tag="t0", name="t0")
            nc.vector.tensor_scalar_mul(
                out=t0, in0=pobA[:, 0:256], scalar1=rcp_sb[:, c + 0:c + 1])
            nc.vector.scalar_tensor_tensor(
                out=pp, in0=pobA[:, 256:512], scalar=rcp_sb[:, c + 1:c + 2],
                in1=t0, op0=mybir.AluOpType.mult, op1=mybir.AluOpType.add)

        def emit_outproj_second(nt, rcp_sb, pp):
            # heads 2,3 + accumulate partial, then store
            pobB = POB.tile([128, 512], f32, tag="pob", name="pobB")
            sl = slice(nt * 128, (nt + 1) * 128)
            for hh in range(2):
                nc.tensor.matmul(pobB[:, 256 * hh:256 * hh + 256],
                                 oTS[0:64, 2 + hh, sl], wo_sb[0:64, 2 + hh, :],
                                 start=True, stop=True)
            c = 4 * (nt % 4)
            t1 = SM.tile([128, 256], f32, tag="t1", name="t1")
            ot = OS.tile([128, 256], f32, tag="ot", name="ot")
            nc.vector.scalar_tensor_tensor(
                out=t1, in0=pobB[:, 0:256], scalar=rcp_sb[:, c + 2:c + 3],
                in1=pp, op0=mybir.AluOpType.mult, op1=mybir.AluOpType.add)
            nc.vector.scalar_tensor_tensor(
                out=ot, in0=pobB[:, 256:512], scalar=rcp_sb[:, c + 3:c + 4],
                in1=t1, op0=mybir.AluOpType.mult, op1=mybir.AluOpType.add)
            nc.sync.dma_start(out=out[sl, :], in_=ot)

        # ---------------- context path (serial prefix) --------------------
        for t in range(8, MT):
            nc.sync.dma_start(out=cs[:, t, :], in_=cr[:, t, :])
        for t in range(4, NT):
            nc.sync.dma_start(out=xs[:, t, :], in_=xr[:, t, :])

        for t in range(4):
            emit_transpose(cs, cT, t)
        for t in range(4):
            emit_transpose(xs, xT, t)
        emit_proj(wk_sb, bkc_sb, cT, kT, 0, 0)
        emit_proj(wq_sb, bqc_sb, xT, qT, 0, 0)
        for t in range(4, 8):
            emit_transpose(cs, cT, t)
        emit_proj(wk_sb, bkc_sb, cT, kT, 0, 1)
        nc.vector.tensor_copy(
            vS[:, :, :, DH],
            cst_sb[:, 0:64].rearrange("p (a b) -> p a b", a=MT))
        for mt in range(3):
            emit_vproj(mt)

        # ---------------- attention blocks --------------------------------
        dn_tiles = {}
        rcp_tiles = {}
        pp_tiles = {}
        for ii in range(NB):
            dn_sb = SM.tile([4, 512], bf16, tag="dn", name=f"dn{ii}")
            dn_tiles[ii] = dn_sb
            rcp_tiles[ii] = SM.tile([128, 16], f32, tag="rcp", name=f"rcp{ii}")
            if ii == NB - 1:
                # rows 2:4 are read (as junk) by the early rcp before block
                # (3,1) writes them; keep them finite for the reciprocal
                nc.vector.memset(dn_sb, 1.0)
            pp_tiles[ii] = [
                SM.tile([128, 256], f32, tag="pp", bufs=8, name=f"pp{ii}_{j}")
                for j in range(4)]
            for hp in range(2):
                h0, h1 = 2 * hp, 2 * hp + 1
                av0 = PSV.tile([128, 512], f32, tag="av", name="av0")
                av1 = PSV.tile([128, 512], f32, tag="av", name="av1")

                def emit_av(j2, e2, av0=av0, av1=av1, h0=h0, h1=h1):
                    nc.tensor.matmul(
                        av0[0:DH + 1, :], vS[:, j2, h0, :], e2[:, 0:512],
                        start=(j2 == 0), stop=(j2 == MT - 1),
                        skip_group_check=True)
                    nc.tensor.matmul(
                        av1[0:DH + 1, :], vS[:, j2, h1, :], e2[:, 512:1024],
                        start=(j2 == 0), stop=(j2 == MT - 1),
                        skip_group_check=True)

                # extra PE/DVE work injected into this block's jj loop
                extras = {}

                def add_extra(jj, fn, extras=extras):
                    extras.setdefault(jj, []).append(fn)

                if ii == 0 and hp == 0:
                    for t_i in range(8, MT):
                        add_extra((t_i - 8) // 2,
                                  lambda t_i=t_i: emit_transpose(cs, cT, t_i))
                    add_extra(4, lambda: emit_proj(wk_sb, bkc_sb, cT, kT, 0, 2))
                    add_extra(8, lambda: emit_proj(wk_sb, bkc_sb, cT, kT, 0, 3))
                    for mt_i in range(3, MT):
                        add_extra(mt_i - 3, lambda mt_i=mt_i: emit_vproj(mt_i))
                    add_extra(11, lambda: emit_proj(wk_sb, bkc_sb, cT, kT, 1, 0))
                    add_extra(13, lambda: emit_proj(wq_sb, bqc_sb, xT, qT, 1, 0))
                if ii == 0 and hp == 1:
                    for b_i in range(1, 4):
                        add_extra(b_i - 1, lambda b_i=b_i: emit_proj(
                            wk_sb, bkc_sb, cT, kT, 1, b_i))
                if hp == 0 and ii > 0:
                    pii = ii - 1
                    add_extra(2, lambda pii=pii: emit_rcp(
                        dn_tiles[pii], rcp_tiles[pii]))
                    for nt_i in range(4):
                        add_extra(4 + 3 * nt_i,
                                  lambda pii=pii, nt_i=nt_i: emit_outproj_first(
                                      4 * pii + nt_i, rcp_tiles[pii],
                                      pp_tiles[pii][nt_i]))
                        add_extra(5 + 3 * nt_i,
                                  lambda pii=pii, nt_i=nt_i: emit_outproj_second(
                                      4 * pii + nt_i, rcp_tiles[pii],
                                      pp_tiles[pii][nt_i]))
                if hp == 1 and ii < NB - 1:
                    nxt = ii + 1
                    for t_i in range(4):
                        add_extra(1 + 2 * t_i,
                                  lambda nxt=nxt, t_i=t_i: emit_transpose(
                                      xs, xT, 4 * nxt + t_i))
                    add_extra(9, lambda nxt=nxt: emit_proj(
                        wq_sb, bqc_sb, xT, qT, 0, nxt))
                    add_extra(11, lambda nxt=nxt: emit_proj(
                        wq_sb, bqc_sb, xT, qT, 1, nxt))
                if hp == 1 and ii == NB - 1:
                    add_extra(6, lambda: emit_rcp(dn_tiles[3], rcp_tiles[3]))
                    for nt_i in range(4):
                        add_extra(8 + 2 * nt_i,
                                  lambda nt_i=nt_i: emit_outproj_first(
                                      12 + nt_i, rcp_tiles[3],
                                      pp_tiles[3][nt_i]))

                SKEW = 3
                exq = []
                for jj in range(MT):
                    sp = PSS.tile([128, 1024], f32, tag="sim", name="sp")
                    nc.tensor.matmul(
                        sp[:, 0:512],
                        kT[0:64, hp, jj * 128:(jj + 1) * 128],
                        qT[0:64, hp, ii * 512:(ii + 1) * 512],
                        start=True, stop=True)
                    nc.tensor.matmul(
                        sp[:, 512:1024],
                        kT[64:128, hp, jj * 128:(jj + 1) * 128],
                        qT[64:128, hp, ii * 512:(ii + 1) * 512],
                        start=True, stop=True)
                    ex = EX.tile([128, 1024], bf16, tag="exp", name="ex")
                    nc.scalar.activation(ex, sp, mybir.ActivationFunctionType.Exp)
                    exq.append((jj, ex))
                    for fn in extras.get(jj, ()):
                        fn()
                    if len(exq) > SKEW:
                        j2, e2 = exq.pop(0)
                        emit_av(j2, e2)
                for j2, e2 in exq:
                    emit_av(j2, e2)

                # denominators + unnormalized attn out to SBUF, denom rows to dn
                sli = slice(ii * 512, (ii + 1) * 512)
                nc.vector.tensor_copy(oTS[0:DH + 1, h0, sli], av0[0:DH + 1, :])
                nc.vector.tensor_copy(oTS[0:DH + 1, h1, sli], av1[0:DH + 1, :])
                nc.sync.dma_start(out=dn_sb[h0:h0 + 1, :],
                                  in_=oTS[DH:DH + 1, h0, sli])
                nc.sync.dma_start(out=dn_sb[h1:h1 + 1, :],
                                  in_=oTS[DH:DH + 1, h1, sli])

        # tail: second halves (heads 2,3) of the last ii's out-projection
        rcp23 = SM.tile([128, 16], f32, tag="rcp", name="rcp23")
        emit_rcp(dn_tiles[3], rcp23)
        for nt_i in range(4):
            emit_outproj_second(12 + nt_i, rcp23, pp_tiles[3][nt_i])

    nc.finalize()
    return nc


def _get_nc():
    if "nc" not in _CACHE:
        _CACHE["nc"] = _build()
    return _CACHE["nc"]


def _make_in_maps(x, context, Wq, bq, Wkv, bkv, Wo, bo):
    f = np.float32
    b16 = ml_dtypes.bfloat16
    inner = HEADS * DH
    cstv = np.ones((128, 256), dtype=b16)
    cstv[:, 128:256] = np.eye(128, dtype=np.float32).astype(b16)
    in_maps = []
    for c in range(NCORES):
        b, g = divmod(c, 2)
        sl = slice(g * E, (g + 1) * E)
        slv = slice(inner + g * E, inner + (g + 1) * E)
        woT = np.ascontiguousarray(np.asarray(Wo)[:, sl].T, dtype=f)   # [E, OD]
        bq_l = (np.asarray(bq, dtype=f)[sl] * SCALE).reshape(KD, 128).T
        bk_l = np.asarray(bkv, dtype=f)[sl].reshape(KD, 128).T
        in_maps.append({
            "x": np.ascontiguousarray(np.asarray(x[b], dtype=f)).astype(b16),
            "cx": np.ascontiguousarray(np.asarray(context[b], dtype=f)).astype(b16),
            "wq": np.ascontiguousarray((np.asarray(Wq, dtype=f)[sl] * SCALE).T).astype(b16),
            "wk": np.ascontiguousarray(np.asarray(Wkv, dtype=f)[sl].T).astype(b16),
            "wv": np.ascontiguousarray(np.asarray(Wkv, dtype=f)[slv].T).astype(b16),
            "wo": woT.reshape(EH, DH, OD).astype(b16),
            "bqc": np.ascontiguousarray(bq_l),
            "bkc": np.ascontiguousarray(bk_l),
            "bv": np.tile(np.asarray(bkv, dtype=f)[slv].reshape(1, E),
                          (128, 1)).astype(b16),
            "cst": cstv,
        })
    return in_maps


def _run(in_maps, trace=False, tmpdir=None):
    nc = _get_nc()
    return run_bass_kernel_spmd(nc, in_maps, list(range(NCORES)),
                                trace=trace, tmpdir=tmpdir)


def kernel(x, context, Wq, bq, Wkv, bkv, Wo, bo):
    in_maps = _make_in_maps(x, context, Wq, bq, Wkv, bkv, Wo, bo)
    res = _run(in_maps)
    parts = [r["out"] for r in res.results]
    bo_f = np.asarray(bo, dtype=np.float32)
    full = np.stack([parts[2 * b] + parts[2 * b + 1] + bo_f for b in range(B)])
    return full.astype(np.float32)
